# revision 20
# baseline (speedup 1.0000x reference)
"""Data-parallel GeneratedTreeClassifier forward on 8 NeuronCores (Bass/Tile).

Shards the batch dim of x (16384 -> 8 x 2048) across cores, replicates the
small tree params, runs a hand-written Bass/Tile kernel per core, and
gathers the full [16384, 512] output.

Math restructure (per tree t, decisions i = 0..3, r = 1/(4 + d3)):
  leaf_probs = r * [1, d0, 1-d0, d1, 1-d1, d2, 1-d2, d3]
  out = S[c] + e @ [G0; G1; G2; G3']      (K = 256, e_i = d_i r)
  G3' = M_t7 - C_t/4,  S[c] = sum_t C_t[c]/4   (rank-1, added on HOST)
  where M = softmax(leaf_logits) * w_tree; C/G row-combines of M are all
  precomputed on the host (cg = [G0;G1;G2;G3']).

Both matmuls run in fp8e4m3 with perf_mode=DoubleRow (K packed 2/cell), so
mm1 (K=512) is 4 matmuls and mm2 (K=256) is 4 matmuls per 512-row group.
fp8 scaling: T tiles hold 4e (rf4 = 1 - 0.2*sigmoid = 4r), cg is scaled
x16 on the host, and the PSUM evacuation copies divide by 64.

Per-core device graph (4 groups of 512 batch rows, software-pipelined so
mm1 of group g+1 runs on the PE while group g's ACT/DVE chain computes):
  zT   = W @ x^T                 (PE, DoubleRow fp8)
  d    = sigmoid(zT + bias)      (ACT, bias per-partition)
  rf4  = 1 - 0.2*sigmoid(z3 + ln1.25)   (exact: = 4/(4+d3); both halves
         written by partition-shifted DVE tensor_scalar ops)
  Td   = [d01; d23] * rf4        (DVE, fp8 out, lhsT of mm2)
  out  = Td @ cgd / 64           (PE DoubleRow; evacuate halves DVE/ACT)
All input DMAs are issued on the sync ring in strict priority order; the
PE HAM clock-gate is pre-warmed with dummy matmuls during the DMA wait.
"""
import numpy as np
import ml_dtypes
from contextlib import ExitStack

import concourse.bass as bass
import concourse.tile as tile
from concourse import bacc, mybir

INPUT_DIM = 512
N_CLASSES = 512
N_TREES = 64
N_LEAVES = 8
N_INTERNAL = 7
PPT = N_INTERNAL * (INPUT_DIM + 1) + N_LEAVES * N_CLASSES
BATCH = 16384
N_CORES = 8
BSH = BATCH // N_CORES          # 2048 rows per core
NG = 4                          # 4 groups of 512 rows
NW = N_INTERNAL * INPUT_DIM

F32 = mybir.dt.float32
BF16 = mybir.dt.bfloat16
F8 = mybir.dt.float8e4
F8NP = ml_dtypes.float8_e4m3
DR = mybir.MatmulPerfMode.DoubleRow


def _emit(ctx: ExitStack, tc, xt, pbf, cgb, pf32, out):
    nc = tc.nc
    AF = mybir.ActivationFunctionType
    MUL = mybir.AluOpType.mult
    ADD = mybir.AluOpType.add

    const = ctx.enter_context(tc.tile_pool(name="const", bufs=1))

    # fp8 params: wt8[p, kk, ko, j] = W^T[kk*256+ko*128+p, j]  (j = i*64+t)
    wt8 = const.tile([128, 2, 2, 256], F8)
    # cgd[p, ko, c] = cg rows (ko*128+p), bf16 (mm2 runs in bf16)
    cgd = const.tile([128, 2, 512], BF16)
    pf = const.tile([128, 3], F32)
    # x^T fp8: xT[p, g, kk, ko, b] = x[g*512 + b, kk*256 + ko*128 + p]
    xT = const.tile([128, NG, 2, 2, 512], F8)

    # Input DMAs: one ring (sync/HWDGE), strict FIFO = priority order.
    nc.sync.dma_start(wt8[:], pbf[:].rearrange(
        "p (kk ko j) -> p kk ko j", kk=2, ko=2))
    nc.sync.dma_start(xT[:, 0], xt[0:128, :].rearrange(
        "p (kk ko b) -> p kk ko b", kk=2, ko=2))
    nc.sync.dma_start(pf[:], pf32[:])
    nc.sync.dma_start(xT[:, 1], xt[128:256, :].rearrange(
        "p (kk ko b) -> p kk ko b", kk=2, ko=2))
    nc.sync.dma_start(cgd[:], cgb[:].rearrange("p (ko c) -> p ko c", ko=2))
    nc.sync.dma_start(xT[:, 2], xt[256:384, :].rearrange(
        "p (kk ko b) -> p kk ko b", kk=2, ko=2))
    nc.sync.dma_start(xT[:, 3], xt[384:512, :].rearrange(
        "p (kk ko b) -> p kk ko b", kk=2, ko=2))

    dpool = ctx.enter_context(tc.tile_pool(name="work", bufs=2))
    epool = ctx.enter_context(tc.tile_pool(name="eT", bufs=2))
    opool = ctx.enter_context(tc.tile_pool(name="osb", bufs=2))
    zpp = ctx.enter_context(tc.tile_pool(name="zps", bufs=2, space="PSUM"))
    opp = ctx.enter_context(tc.tile_pool(name="ops", bufs=2, space="PSUM"))

    # Warm the PE HAM clock-gate while the input DMAs are in flight: a few
    # dependency-free matmuls on zeroed scratch, written into group 0's zt
    # tile (the first real matmul restarts the accumulation with start=True).
    scratch = const.tile([128, 512], BF16)
    nc.gpsimd.memset(scratch[:], 0.0)
    warm = zpp.tile([128, 2, 512], F32, tag="zt")
    for _ in range(4):
        nc.tensor.matmul(warm[:, 0, :], lhsT=scratch[:, 0:128], rhs=scratch[:],
                         start=True, stop=True)

    def mm1(g, zt=None):
        # zT[j, b] = sum_d W[j, d] x[b, d]    j = i*64 + t, i-major
        if zt is None:
            zt = zpp.tile([128, 2, 512], F32, tag="zt")
        for jb in range(2):
            for kk in range(2):
                nc.tensor.matmul(zt[:, jb, :],
                                 lhsT=wt8[:, kk, :, jb * 128:(jb + 1) * 128],
                                 rhs=xT[:, g, kk],
                                 perf_mode=DR,
                                 start=(kk == 0), stop=(kk == 1))
        return zt

    zt = mm1(0, zt=warm)
    for g in range(NG):
      # Stagger each group in the scheduler's simulated clock so its greedy
      # per-engine ordering can't interleave group g+1's chain ops ahead of
      # group g's (pure ordering hint: adds no waits on hardware).
      with tc.tile_wait_until(0.0025 * g, enable=g > 0):
        # r = 1/(4 + d3) = 1/4 - sigmoid(z3 + ln 1.25)/20   (exact identity)
        s3 = dpool.tile([128, 512], F32, tag="s3")
        nc.scalar.activation(s3[64:128, :], zt[64:128, 1, :], AF.Sigmoid,
                             bias=pf[64:128, 2:3])
        rf = dpool.tile([128, 512], BF16, tag="rf")
        nc.vector.tensor_scalar(rf[64:128, :], s3[64:128, :], -0.05, 0.25,
                                op0=MUL, op1=ADD)
        nc.vector.tensor_scalar(rf[0:64, :], s3[64:128, :], -0.05, 0.25,
                                op0=MUL, op1=ADD)       # partition-shifted
        d0 = dpool.tile([128, 512], BF16, tag="d0")
        nc.scalar.activation(d0[:], zt[:, 0, :], AF.Sigmoid, bias=pf[:, 0:1])
        # T0 = d01 * r: ready before d1's sigmoid -> mm2's T0 half can start
        T0 = epool.tile([128, 512], BF16, tag="T0")
        nc.vector.tensor_tensor(T0[:], d0[:], rf[:], op=MUL)
        d1 = dpool.tile([128, 512], BF16, tag="d1")
        nc.scalar.activation(d1[:], zt[:, 1, :], AF.Sigmoid, bias=pf[:, 1:2])
        T1 = epool.tile([128, 512], BF16, tag="T1")
        nc.vector.tensor_tensor(T1[:], d1[:], rf[:], op=MUL)

        # keep the PE fed: next group's mm1 goes ahead of this group's mm2
        if g + 1 < NG:
            zt = mm1(g + 1)

        # out = T0 @ cg0 + T1 @ cg1 per batch tile (bf16).  All T0 matmuls
        # are issued first so the PE starts before d1/T1 are ready.
        osb = opool.tile([128, 4, 512], BF16, tag="osb")
        ops_a = opp.tile([128, 2, 512], F32, tag="o")
        ops_b = opp.tile([128, 2, 512], F32, tag="o")
        opsh = [ops_a, ops_b]
        for bt in range(4):
            nc.tensor.matmul(opsh[bt // 2][:, bt % 2, :],
                             lhsT=T0[:, bt * 128:(bt + 1) * 128],
                             rhs=cgd[:, 0, :], start=True, stop=False)
        for h in range(2):
            for bt in (2 * h, 2 * h + 1):
                nc.tensor.matmul(opsh[h][:, bt % 2, :],
                                 lhsT=T1[:, bt * 128:(bt + 1) * 128],
                                 rhs=cgd[:, 1, :], start=False, stop=True)
            # evacuate: each half split across DVE+ACT (balances engine load)
            nc.vector.tensor_copy(osb[:, 2 * h, :], opsh[h][:, 0, :])
            nc.scalar.copy(osb[:, 2 * h + 1, :], opsh[h][:, 1, :])
            # out row = g*128 + p, col = bt*512 + c  (host un-permutes);
            # the last group stores per half so the final bytes leave early
            if g == NG - 1:
                nc.gpsimd.dma_start(
                    out[g * 128:(g + 1) * 128, h * 1024:(h + 1) * 1024],
                    osb[:, 2 * h:2 * h + 2, :].rearrange("p bt c -> p (bt c)"))
        if g < NG - 1:
            nc.gpsimd.dma_start(out[g * 128:(g + 1) * 128, :],
                                osb[:].rearrange("p bt c -> p (bt c)"))


_NC = None
_RUNNER = None


def _get_nc():
    global _NC
    if _NC is None:
        nc = bacc.Bacc("TRN2", target_bir_lowering=False, debug=False)
        xt = nc.dram_tensor("xt", [512, 2048], F8, kind="ExternalInput")
        pbf = nc.dram_tensor("pbf", [128, 1024], F8, kind="ExternalInput")
        cgb = nc.dram_tensor("cgb", [128, 1024], BF16, kind="ExternalInput")
        pf32 = nc.dram_tensor("pf32", [128, 3], F32, kind="ExternalInput")
        out = nc.dram_tensor("out", [512, 2048], BF16, kind="ExternalOutput")
        with tile.TileContext(nc) as tc, ExitStack() as ctx:
            _emit(ctx, tc, xt.ap(), pbf.ap(), cgb.ap(), pf32.ap(), out.ap())
        nc.compile()
        _NC = nc
    return _NC


def _get_runner():
    """Build the sharded PJRT executable ONCE (jit + NEFF compile are cached
    across kernel() calls; run_bass_kernel_spmd would re-trace every call)."""
    global _RUNNER
    if _RUNNER is None:
        import jax
        import jax.numpy as jnp
        from jax.sharding import Mesh, PartitionSpec, NamedSharding
        from jax.experimental.shard_map import shard_map
        from concourse import bass2jax

        nc = _get_nc()
        bass2jax.install_neuronx_cc_hook()

        part_name = (nc.partition_id_tensor.name
                     if nc.partition_id_tensor else None)
        in_names, out_names, out_avals = [], [], []
        for alloc in nc.m.functions[0].allocations:
            if not isinstance(alloc, mybir.MemoryLocationSet):
                continue
            name = alloc.memorylocations[0].name
            if alloc.kind == "ExternalInput":
                if name != part_name:
                    in_names.append(name)
            elif alloc.kind == "ExternalOutput":
                out_names.append(name)
                out_avals.append(jax.core.ShapedArray(
                    tuple(alloc.tensor_shape), mybir.dt.np(alloc.dtype)))
        n_params = len(in_names)
        all_names = tuple(in_names) + tuple(out_names)
        if part_name is not None:
            all_names = all_names + (part_name,)
        donate = tuple(range(n_params, n_params + len(out_names)))

        def _body(*args):
            operands = list(args)
            if part_name is not None:
                operands.append(bass2jax.partition_id_tensor())
            outs = bass2jax._bass_exec_p.bind(
                *operands,
                out_avals=tuple(out_avals),
                in_names=all_names,
                out_names=tuple(out_names),
                lowering_input_output_aliases=(),
                sim_require_finite=True,
                sim_require_nnan=True,
                nc=nc,
            )
            return tuple(outs)

        devices = jax.devices()[:N_CORES]
        mesh = Mesh(np.asarray(devices), ("core",))
        spec = PartitionSpec("core")
        fn = jax.jit(
            shard_map(_body, mesh=mesh,
                      in_specs=(spec,) * (n_params + len(out_names)),
                      out_specs=(spec,) * len(out_names), check_rep=False),
            donate_argnums=donate, keep_unused=True)
        zmk = jax.jit(
            lambda: jnp.zeros((N_CORES * 512, 2048), ml_dtypes.bfloat16),
            out_shardings=NamedSharding(mesh, spec))
        _RUNNER = (fn, zmk, in_names)
    return _RUNNER


def _host_prep(x, tree_params, tree_weights):
    """Host-side: transpose/group x (fp8), pack replicated params, and fold
    the leaf-distribution combination matrices (incl. softmax) plus the
    rank-1 output shift S into precomputed arrays."""
    x = np.asarray(x, np.float32)
    # xt[(g p), (kk ko b)] = x_core[g*512 + b, kk*256 + ko*128 + p], per core
    xt = np.ascontiguousarray(
        x.reshape(N_CORES, NG, 512, 2, 2, 128).transpose(0, 1, 5, 3, 4, 2)
    ).reshape(N_CORES * 512, 2048).astype(F8NP)

    p = np.asarray(tree_params, np.float32)[0].reshape(N_TREES, PPT)
    w4 = p[:, :NW].reshape(N_TREES, N_INTERNAL, INPUT_DIM)[:, :4, :]
    wj = w4.transpose(1, 0, 2).reshape(256, INPUT_DIM)      # j = i*64 + t
    # wt8[p, kk, ko, j] -> [128, 1024]
    wt8 = np.ascontiguousarray(
        wj.T.reshape(2, 2, 128, 256).transpose(2, 0, 1, 3)).reshape(128, 1024)

    ll = p[:, NW + N_INTERNAL:].reshape(N_TREES, N_LEAVES, N_CLASSES)
    e = np.exp(ll - ll.max(axis=-1, keepdims=True))
    M = e / e.sum(axis=-1, keepdims=True)                   # softmax [T, L, C]
    M = M * np.asarray(tree_weights, np.float32)[0][:, None, None]
    C_ = M[:, 0] + M[:, 2] + M[:, 4] + M[:, 6]              # [T, C]
    G0 = M[:, 1] - M[:, 2]
    G1 = M[:, 3] - M[:, 4]
    G2 = M[:, 5] - M[:, 6]
    G3 = M[:, 7] - C_ * 0.25
    cg0 = np.concatenate([G0, G1], 0)                       # [128, C]
    cg1 = np.concatenate([G2, G3], 0)
    # cgb[p, (ko c)] = cg_ko[p, c]  -> [128, 1024] bf16
    cgb = np.ascontiguousarray(
        np.stack([cg0, cg1], axis=1).reshape(128, 1024)).astype(
            ml_dtypes.bfloat16)
    pbf = wt8.astype(F8NP)                                  # [128, 1024]

    bias = p[:, NW:NW + N_INTERNAL][:, :4].T.reshape(256)   # j-major
    pf32 = np.zeros((128, 3), np.float32)
    pf32[:, 0] = bias[0:128]
    pf32[:, 1] = bias[128:256]
    pf32[64:128, 2] = bias[192:256] + np.float32(np.log(1.25))

    S = C_.sum(axis=0) * 0.25                               # [C] host shift
    return xt, pbf, cgb, pf32, S


def _unpermute(outd, S):
    """outd [N_CORES*512, 2048] with row g*128+p, col bt*512+c ->
    full [16384, 512] f32 plus the rank-1 shift."""
    o = outd.reshape(N_CORES, NG, 128, 4, 512).transpose(0, 1, 3, 2, 4)
    return np.ascontiguousarray(o).reshape(BATCH, N_CLASSES).astype(
        np.float32) + S[None, :]


def kernel(x: np.ndarray, tree_params: np.ndarray,
           tree_weights: np.ndarray) -> np.ndarray:
    fn, zmk, in_names = _get_runner()
    xt, pbf, cgb, pf32, S = _host_prep(x, tree_params, tree_weights)
    reps = {"xt": xt,
            "pbf": np.concatenate([pbf] * N_CORES, 0),
            "cgb": np.concatenate([cgb] * N_CORES, 0),
            "pf32": np.concatenate([pf32] * N_CORES, 0)}
    args = [reps[n] for n in in_names] + [zmk()]
    outs = fn(*args)
    return _unpermute(np.asarray(outs[0]), S)


# revision 29
# speedup vs baseline: 1.0796x; 1.0796x over previous
"""Data-parallel GeneratedTreeClassifier forward on 8 NeuronCores (Bass/Tile).

Shards the batch dim of x (16384 -> 8 x 2048) across cores, replicates the
small tree params, runs a hand-written Bass/Tile kernel per core, and
gathers the full [16384, 512] output.

Math restructure (per tree t, decisions i = 0..3, r = 1/(4 + d3)):
  leaf_probs = r * [1, d0, 1-d0, d1, 1-d1, d2, 1-d2, d3]
  out = S[c] + e @ [G0; G1; G2; G3']      (K = 256, e_i = d_i r)
  G3' = M_t7 - C_t/4,  S[c] = sum_t C_t[c]/4   (rank-1, added on HOST)
  where M = softmax(leaf_logits) * w_tree; C/G row-combines of M are all
  precomputed on the host (cg = [G0;G1;G2;G3']).

Both matmuls run in fp8e4m3 with perf_mode=DoubleRow (K packed 2/cell), so
mm1 (K=512) is 4 matmuls and mm2 (K=256) is 4 matmuls per 512-row group.
fp8 scaling: T tiles hold 4e (rf4 = 1 - 0.2*sigmoid = 4r), cg is scaled
x16 on the host, and the PSUM evacuation copies divide by 64.

Per-core device graph (4 groups of 512 batch rows, software-pipelined so
mm1 of group g+1 runs on the PE while group g's ACT/DVE chain computes):
  zT   = W @ x^T                 (PE, DoubleRow fp8)
  d    = sigmoid(zT + bias)      (ACT, bias per-partition)
  rf4  = 1 - 0.2*sigmoid(z3 + ln1.25)   (exact: = 4/(4+d3); both halves
         written by partition-shifted DVE tensor_scalar ops)
  Td   = [d01; d23] * rf4        (DVE, fp8 out, lhsT of mm2)
  out  = Td @ cgd / 64           (PE DoubleRow; evacuate halves DVE/ACT)
All input DMAs are issued on the sync ring in strict priority order; the
PE HAM clock-gate is pre-warmed with dummy matmuls during the DMA wait.
"""
import numpy as np
import ml_dtypes
from contextlib import ExitStack

import concourse.bass as bass
import concourse.tile as tile
from concourse import bacc, mybir

INPUT_DIM = 512
N_CLASSES = 512
N_TREES = 64
N_LEAVES = 8
N_INTERNAL = 7
PPT = N_INTERNAL * (INPUT_DIM + 1) + N_LEAVES * N_CLASSES
BATCH = 16384
N_CORES = 8
BSH = BATCH // N_CORES          # 2048 rows per core
NG = 4                          # 4 groups of 512 rows
NW = N_INTERNAL * INPUT_DIM

F32 = mybir.dt.float32
BF16 = mybir.dt.bfloat16
F8 = mybir.dt.float8e4
F8NP = ml_dtypes.float8_e4m3
DR = mybir.MatmulPerfMode.DoubleRow


def _emit(ctx: ExitStack, tc, xt, pbf, pf32, out):
    nc = tc.nc
    AF = mybir.ActivationFunctionType
    MUL = mybir.AluOpType.mult
    ADD = mybir.AluOpType.add

    const = ctx.enter_context(tc.tile_pool(name="const", bufs=1))

    # fp8 params: wt8[p, kk, ko, j] = W^T[kk*256+ko*128+p, j]  (j = i*64+t)
    wt8 = const.tile([128, 2, 2, 256], F8)
    # cgd[p, ko, c] = 16 * cg rows (ko*128+p), fp8 (mm2 runs DoubleRow fp8)
    cgd = const.tile([128, 2, 512], F8)
    pf = const.tile([128, 3], F32)
    # x^T fp8: xT[p, g, kk, ko, b] = x[g*512 + b, kk*256 + ko*128 + p]
    xT = const.tile([128, NG, 2, 2, 512], F8)

    # Input DMAs: one ring (sync/HWDGE), strict FIFO = priority order.
    nc.sync.dma_start(wt8[:], pbf[:, 0:1024].rearrange(
        "p (kk ko j) -> p kk ko j", kk=2, ko=2))
    nc.sync.dma_start(xT[:, 0], xt[0:128, :].rearrange(
        "p (kk ko b) -> p kk ko b", kk=2, ko=2))
    nc.sync.dma_start(pf[:], pf32[:])
    nc.sync.dma_start(xT[:, 1], xt[128:256, :].rearrange(
        "p (kk ko b) -> p kk ko b", kk=2, ko=2))
    nc.sync.dma_start(cgd[:], pbf[:, 1024:2048].rearrange(
        "p (ko c) -> p ko c", ko=2))
    nc.sync.dma_start(xT[:, 2], xt[256:384, :].rearrange(
        "p (kk ko b) -> p kk ko b", kk=2, ko=2))
    nc.sync.dma_start(xT[:, 3], xt[384:512, :].rearrange(
        "p (kk ko b) -> p kk ko b", kk=2, ko=2))

    dpool = ctx.enter_context(tc.tile_pool(name="work", bufs=2))
    epool = ctx.enter_context(tc.tile_pool(name="eT", bufs=2))
    opool = ctx.enter_context(tc.tile_pool(name="osb", bufs=2))
    zpp = ctx.enter_context(tc.tile_pool(name="zps", bufs=2, space="PSUM"))
    opp = ctx.enter_context(tc.tile_pool(name="ops", bufs=2, space="PSUM"))

    # Warm the PE HAM clock-gate while the input DMAs are in flight: a few
    # dependency-free matmuls on zeroed scratch, written into group 0's zt
    # tile (the first real matmul restarts the accumulation with start=True).
    scratch = const.tile([128, 512], BF16)
    nc.gpsimd.memset(scratch[:], 0.0)
    warm = zpp.tile([128, 2, 512], F32, tag="zt")
    for _ in range(5):
        nc.tensor.matmul(warm[:, 0, :], lhsT=scratch[:, 0:128], rhs=scratch[:],
                         start=True, stop=True)

    def mm1(g, zt=None):
        # zT[j, b] = sum_d W[j, d] x[b, d]    j = i*64 + t, i-major
        if zt is None:
            zt = zpp.tile([128, 2, 512], F32, tag="zt")
        for jb in range(2):
            for kk in range(2):
                nc.tensor.matmul(zt[:, jb, :],
                                 lhsT=wt8[:, kk, :, jb * 128:(jb + 1) * 128],
                                 rhs=xT[:, g, kk],
                                 perf_mode=DR,
                                 start=(kk == 0), stop=(kk == 1))
        return zt

    zt = mm1(0, zt=warm)
    for g in range(NG):
        # r = 1/(4 + d3) = 1/4 - sigmoid(z3 + ln 1.25)/20   (exact identity;
        # rf holds 4r so Td = 4e stays in fp8 range)
        s3 = dpool.tile([128, 512], F32, tag="s3")
        nc.scalar.activation(s3[64:128, :], zt[64:128, 1, :], AF.Sigmoid,
                             bias=pf[64:128, 2:3])
        rf = dpool.tile([128, 512], BF16, tag="rf")
        nc.vector.tensor_scalar(rf[64:128, :], s3[64:128, :], -0.2, 1.0,
                                op0=MUL, op1=ADD)
        nc.vector.tensor_scalar(rf[0:64, :], s3[64:128, :], -0.2, 1.0,
                                op0=MUL, op1=ADD)       # partition-shifted
        d0 = dpool.tile([128, 512], BF16, tag="d0")
        nc.scalar.activation(d0[:], zt[:, 0, :], AF.Sigmoid, bias=pf[:, 0:1])
        d1 = dpool.tile([128, 512], BF16, tag="d1")
        nc.scalar.activation(d1[:], zt[:, 1, :], AF.Sigmoid, bias=pf[:, 1:2])

        # Td[:, ko, :] = 4 * e rows (ko*128+p)  -> fp8 lhsT for mm2
        Td = epool.tile([128, 2, 512], F8, tag="Td")
        nc.vector.tensor_tensor(Td[:, 0, :], d0[:], rf[:], op=MUL)
        nc.vector.tensor_tensor(Td[:, 1, :], d1[:], rf[:], op=MUL)

        # keep the PE fed: next group's mm1 goes ahead of this group's mm2
        if g + 1 < NG:
            zt = mm1(g + 1)

        # out = Td @ cgd / 64  per batch tile; evacuate in halves (DVE/ACT)
        osb = opool.tile([128, 4, 512], BF16, tag="osb")
        for h in range(2):
            ops = opp.tile([128, 2, 512], F32, tag="o")
            for bt in (2 * h, 2 * h + 1):
                bs = slice(bt * 128, (bt + 1) * 128)
                nc.tensor.matmul(ops[:, bt - 2 * h, :],
                                 lhsT=Td[:, :, bs], rhs=cgd[:],
                                 perf_mode=DR, start=True, stop=True)
            if h == 0:
                nc.vector.tensor_scalar(osb[:, 0:2, :], ops[:],
                                        1.0 / 64.0, 0.0, op0=MUL, op1=ADD)
            else:
                nc.scalar.mul(osb[:, 2:4, :], ops[:], 1.0 / 64.0)
            # out row = g*128 + p, col = bt*512 + c  (host un-permutes);
            # the last group stores per half so the final bytes leave early
            if g == NG - 1:
                nc.gpsimd.dma_start(
                    out[g * 128:(g + 1) * 128, h * 1024:(h + 1) * 1024],
                    osb[:, 2 * h:2 * h + 2, :].rearrange("p bt c -> p (bt c)"))
        if g < NG - 1:
            nc.gpsimd.dma_start(out[g * 128:(g + 1) * 128, :],
                                osb[:].rearrange("p bt c -> p (bt c)"))


_NC = None
_RUNNER = None


def _get_nc():
    global _NC
    if _NC is None:
        nc = bacc.Bacc("TRN2", target_bir_lowering=False, debug=False)
        xt = nc.dram_tensor("xt", [512, 2048], F8, kind="ExternalInput")
        pbf = nc.dram_tensor("pbf", [128, 2048], F8, kind="ExternalInput")
        pf32 = nc.dram_tensor("pf32", [128, 3], F32, kind="ExternalInput")
        out = nc.dram_tensor("out", [512, 2048], BF16, kind="ExternalOutput")
        with tile.TileContext(nc) as tc, ExitStack() as ctx:
            _emit(ctx, tc, xt.ap(), pbf.ap(), pf32.ap(), out.ap())
        nc.compile()
        _NC = nc
    return _NC


def _get_runner():
    """Build the sharded PJRT executable ONCE (jit + NEFF compile are cached
    across kernel() calls; run_bass_kernel_spmd would re-trace every call)."""
    global _RUNNER
    if _RUNNER is None:
        import jax
        import jax.numpy as jnp
        from jax.sharding import Mesh, PartitionSpec, NamedSharding
        from jax.experimental.shard_map import shard_map
        from concourse import bass2jax

        nc = _get_nc()
        bass2jax.install_neuronx_cc_hook()

        part_name = (nc.partition_id_tensor.name
                     if nc.partition_id_tensor else None)
        in_names, out_names, out_avals = [], [], []
        for alloc in nc.m.functions[0].allocations:
            if not isinstance(alloc, mybir.MemoryLocationSet):
                continue
            name = alloc.memorylocations[0].name
            if alloc.kind == "ExternalInput":
                if name != part_name:
                    in_names.append(name)
            elif alloc.kind == "ExternalOutput":
                out_names.append(name)
                out_avals.append(jax.core.ShapedArray(
                    tuple(alloc.tensor_shape), mybir.dt.np(alloc.dtype)))
        n_params = len(in_names)
        all_names = tuple(in_names) + tuple(out_names)
        if part_name is not None:
            all_names = all_names + (part_name,)
        donate = tuple(range(n_params, n_params + len(out_names)))

        def _body(*args):
            operands = list(args)
            if part_name is not None:
                operands.append(bass2jax.partition_id_tensor())
            outs = bass2jax._bass_exec_p.bind(
                *operands,
                out_avals=tuple(out_avals),
                in_names=all_names,
                out_names=tuple(out_names),
                lowering_input_output_aliases=(),
                sim_require_finite=True,
                sim_require_nnan=True,
                nc=nc,
            )
            return tuple(outs)

        devices = jax.devices()[:N_CORES]
        mesh = Mesh(np.asarray(devices), ("core",))
        spec = PartitionSpec("core")
        fn = jax.jit(
            shard_map(_body, mesh=mesh,
                      in_specs=(spec,) * (n_params + len(out_names)),
                      out_specs=(spec,) * len(out_names), check_rep=False),
            donate_argnums=donate, keep_unused=True)
        zmk = jax.jit(
            lambda: jnp.zeros((N_CORES * 512, 2048), ml_dtypes.bfloat16),
            out_shardings=NamedSharding(mesh, spec))
        _RUNNER = (fn, zmk, in_names)
    return _RUNNER


def _host_prep(x, tree_params, tree_weights):
    """Host-side: transpose/group x (fp8), pack replicated params, and fold
    the leaf-distribution combination matrices (incl. softmax) plus the
    rank-1 output shift S into precomputed arrays."""
    x = np.asarray(x, np.float32)
    # xt[(g p), (kk ko b)] = x_core[g*512 + b, kk*256 + ko*128 + p], per core
    xt = np.ascontiguousarray(
        x.reshape(N_CORES, NG, 512, 2, 2, 128).transpose(0, 1, 5, 3, 4, 2)
    ).reshape(N_CORES * 512, 2048).astype(F8NP)

    p = np.asarray(tree_params, np.float32)[0].reshape(N_TREES, PPT)
    w4 = p[:, :NW].reshape(N_TREES, N_INTERNAL, INPUT_DIM)[:, :4, :]
    wj = w4.transpose(1, 0, 2).reshape(256, INPUT_DIM)      # j = i*64 + t
    # wt8[p, kk, ko, j] -> [128, 1024]
    wt8 = np.ascontiguousarray(
        wj.T.reshape(2, 2, 128, 256).transpose(2, 0, 1, 3)).reshape(128, 1024)

    ll = p[:, NW + N_INTERNAL:].reshape(N_TREES, N_LEAVES, N_CLASSES)
    e = np.exp(ll - ll.max(axis=-1, keepdims=True))
    M = e / e.sum(axis=-1, keepdims=True)                   # softmax [T, L, C]
    M = M * np.asarray(tree_weights, np.float32)[0][:, None, None]
    C_ = M[:, 0] + M[:, 2] + M[:, 4] + M[:, 6]              # [T, C]
    G0 = M[:, 1] - M[:, 2]
    G1 = M[:, 3] - M[:, 4]
    G2 = M[:, 5] - M[:, 6]
    G3 = M[:, 7] - C_ * 0.25
    cg0 = np.concatenate([G0, G1], 0)                       # [128, C]
    cg1 = np.concatenate([G2, G3], 0)
    # cgd[p, (ko c)] = 16 * cg_ko[p, c]  -> [128, 1024]
    cgd = np.stack([cg0 * 16.0, cg1 * 16.0], axis=1).reshape(128, 1024)
    pbf = np.concatenate([wt8, cgd], axis=1).astype(F8NP)   # [128, 2048]

    bias = p[:, NW:NW + N_INTERNAL][:, :4].T.reshape(256)   # j-major
    pf32 = np.zeros((128, 3), np.float32)
    pf32[:, 0] = bias[0:128]
    pf32[:, 1] = bias[128:256]
    pf32[64:128, 2] = bias[192:256] + np.float32(np.log(1.25))

    S = C_.sum(axis=0) * 0.25                               # [C] host shift
    return xt, pbf, pf32, S


def _unpermute(outd, S):
    """outd [N_CORES*512, 2048] with row g*128+p, col bt*512+c ->
    full [16384, 512] f32 plus the rank-1 shift."""
    o = outd.reshape(N_CORES, NG, 128, 4, 512).transpose(0, 1, 3, 2, 4)
    return np.ascontiguousarray(o).reshape(BATCH, N_CLASSES).astype(
        np.float32) + S[None, :]


def kernel(x: np.ndarray, tree_params: np.ndarray,
           tree_weights: np.ndarray) -> np.ndarray:
    fn, zmk, in_names = _get_runner()
    xt, pbf, pf32, S = _host_prep(x, tree_params, tree_weights)
    reps = {"xt": xt,
            "pbf": np.concatenate([pbf] * N_CORES, 0),
            "pf32": np.concatenate([pf32] * N_CORES, 0)}
    args = [reps[n] for n in in_names] + [zmk()]
    outs = fn(*args)
    return _unpermute(np.asarray(outs[0]), S)
